# revision 10
# baseline (speedup 1.0000x reference)
"""Trainium2 Bass kernel for GQA MHA prefill (S=2048, D=4096, H=32, KVH=8).

Strategy (8 NeuronCores, tensor-parallel over heads):
  - Each core owns 4 query heads + 1 kv head. Host stages transposed,
    head-permuted weight shards so no on-chip transposes are needed for
    the projections: qT/kT/vT come out of the PE directly in [dim, seq]
    layout (seq on the free axis).
  - Head-dim components are permuted (even indices first, odd second) so
    RoPE becomes ops on contiguous partition halves; the permutation is
    applied identically to q and k so logits are unchanged.
  - SDPA runs in transposed layout: logitsT[k, q] = kT.T @ qT per
    128-row k-chunk; exp on ScalarE (no max subtraction needed: logits
    are O(5) by construction); causal masking is structural
    (skip upper blocks + affine_select on diagonal tiles). Softmax
    denominators come free as an extra ones-column in the p@v matmul.
  - o[q, hd] tiles are normalized, PE-transposed to oT and AllToAll'd
    (4 x 1MB per core) from head-sharded to seq-sharded layout, then each
    core computes its 256 output rows against the full wo (streamed).
  - If the mask input is NOT the expected causal mask, a general
    fallback variant applies the mask as data (identity-matmul
    accumulation into PSUM).
"""

import sys

import numpy as np

sys.path.insert(0, "/opt/trn_rl_repo")

S = 2048
D = 4096
H = 32
KVH = 8
HD = 128
NCORES = 8
HL = H // NCORES          # 4 local query heads
DL = HL * HD              # 512 local q dim
SQ = S // NCORES          # 256 output rows per core
GH = HD // 2              # 64 rope pair lanes
KC = S // 128             # 16 key chunks
DC = D // 128             # 32 contraction chunks
NB = S // 512             # 4 seq blocks of 512
QB = S // 512             # 4 q blocks of 512
NEG = -1e9
VST = 130                 # v_sb column stride: 128 hd + 1 ones + 1 pad

_built = {}


def _build(causal: bool):
    import concourse.bass as bass  # noqa: F401
    import concourse.mybir as mybir
    import concourse.tile as tile
    from concourse import bacc
    from concourse.masks import make_identity

    fp32 = mybir.dt.float32
    bf16 = mybir.dt.bfloat16
    AF = mybir.ActivationFunctionType
    OP = mybir.AluOpType

    nc = bacc.Bacc(
        "TRN2",
        target_bir_lowering=False,
        debug=False,
        num_devices=NCORES,
    )
    xT = nc.dram_tensor("xT", [D, S], fp32, kind="ExternalInput")
    wqT = nc.dram_tensor("wqT", [D, DL], fp32, kind="ExternalInput")
    wkT = nc.dram_tensor("wkT", [D, HD], fp32, kind="ExternalInput")
    wvT = nc.dram_tensor("wvT", [D, HD], fp32, kind="ExternalInput")
    cosT = nc.dram_tensor("cosT", [GH, S], fp32, kind="ExternalInput")
    sinT = nc.dram_tensor("sinT", [GH, S], fp32, kind="ExternalInput")
    woT = nc.dram_tensor("woT", [D, D], fp32, kind="ExternalInput")
    if not causal:
        maskT = nc.dram_tensor("maskT", [S, S], fp32, kind="ExternalInput")
    out = nc.dram_tensor("out", [SQ, D], fp32, kind="ExternalOutput")

    rg = [list(range(NCORES))]

    with tile.TileContext(nc) as tc:
        with (
            tc.tile_pool(name="const", bufs=1) as constp,
            tc.tile_pool(name="pers", bufs=1) as pers,
            tc.tile_pool(name="dram", bufs=1, space="DRAM") as dramp,
        ):
            ident = constp.tile([128, 128], fp32, tag="ident")
            make_identity(nc, ident[:])
            c_sb = constp.tile([GH, S], fp32, tag="cos")
            s_sb = constp.tile([GH, S], fp32, tag="sin")
            nc.sync.dma_start(c_sb[:], cosT[:, :])
            nc.sync.dma_start(s_sb[:], sinT[:, :])

            qT_sb = pers.tile([128, HL * S], fp32, tag="qT")
            kT_sb = pers.tile([128, S], fp32, tag="kT")
            v_sb = pers.tile([128, KC * VST], bf16, tag="v")

            a2a_in = [
                dramp.tile(
                    [NCORES * HD, SQ], fp32,
                    tag=f"a2a_in{h}", name=f"a2a_in{h}",
                )
                for h in range(HL)
            ]
            a2a_out = [
                dramp.tile(
                    [NCORES * HD, SQ], fp32,
                    tag=f"a2a_out{h}", name=f"a2a_out{h}",
                )
                for h in range(HL)
            ]

            # ---------------- Stage 1: projections + RoPE ----------------
            with (
                tc.tile_pool(name="s1w", bufs=1) as s1w,
                tc.tile_pool(name="s1x", bufs=4) as s1x,
                tc.tile_pool(name="rope", bufs=2) as ropep,
                tc.tile_pool(name="s1v", bufs=2) as s1v,
                tc.tile_pool(name="ps_q", bufs=1, space="PSUM") as ps_q,
                tc.tile_pool(name="ps_kv", bufs=1, space="PSUM") as ps_kv,
                tc.tile_pool(name="ps_tr", bufs=1, space="PSUM") as ps_tr,
            ):
                wq_sb = s1w.tile([128, DC * DL], fp32, tag="wq")
                wk_sb = s1w.tile([128, DC * HD], fp32, tag="wk")
                wv_sb = s1w.tile([128, DC * HD], fp32, tag="wv")
                nc.sync.dma_start(
                    wq_sb[:], wqT.rearrange("(c p) m -> p c m", p=128)
                )
                nc.sync.dma_start(
                    wk_sb[:], wkT.rearrange("(c p) m -> p c m", p=128)
                )
                nc.sync.dma_start(
                    wv_sb[:], wvT.rearrange("(c p) m -> p c m", p=128)
                )

                def rope(dst, src, col0, ncol):
                    # dst/src: [128, ncol]; rows 0:64 = even comps, 64:128 odd
                    e = src[0:64, :]
                    o = src[64:128, :]
                    c = c_sb[:, col0 : col0 + ncol]
                    s = s_sb[:, col0 : col0 + ncol]
                    t1 = ropep.tile([GH, 512], fp32, tag="t1")
                    t2 = ropep.tile([GH, 512], fp32, tag="t2")
                    nc.vector.tensor_tensor(t1[:, 0:ncol], e, c, OP.mult)
                    nc.vector.tensor_tensor(t2[:, 0:ncol], o, s, OP.mult)
                    nc.vector.tensor_tensor(
                        dst[0:64, :], t1[:, 0:ncol], t2[:, 0:ncol], OP.subtract
                    )
                    t3 = ropep.tile([GH, 512], fp32, tag="t1")
                    t4 = ropep.tile([GH, 512], fp32, tag="t2")
                    nc.vector.tensor_tensor(t3[:, 0:ncol], e, s, OP.mult)
                    nc.vector.tensor_tensor(t4[:, 0:ncol], o, c, OP.mult)
                    nc.vector.tensor_tensor(
                        dst[64:128, :], t3[:, 0:ncol], t4[:, 0:ncol], OP.add
                    )

                for nb in range(NB):
                    qps = [
                        ps_q.tile([128, 512], fp32, tag=f"q{m}", name=f"q{m}")
                        for m in range(HL)
                    ]
                    kps = ps_kv.tile([128, 512], fp32, tag="kk")
                    vps = ps_kv.tile([128, 512], fp32, tag="vv")
                    for c in range(DC):
                        xt = s1x.tile([128, 512], fp32, tag="xt")
                        nc.sync.dma_start(
                            xt[:], xT[c * 128 : (c + 1) * 128, nb * 512 : (nb + 1) * 512]
                        )
                        st = c == 0
                        sp = c == DC - 1
                        for m in range(HL):
                            nc.tensor.matmul(
                                qps[m][:],
                                lhsT=wq_sb[:, c * DL + m * 128 : c * DL + (m + 1) * 128],
                                rhs=xt[:],
                                start=st,
                                stop=sp,
                            )
                        nc.tensor.matmul(
                            kps[:],
                            lhsT=wk_sb[:, c * HD : (c + 1) * HD],
                            rhs=xt[:],
                            start=st,
                            stop=sp,
                        )
                        nc.tensor.matmul(
                            vps[:],
                            lhsT=wv_sb[:, c * HD : (c + 1) * HD],
                            rhs=xt[:],
                            start=st,
                            stop=sp,
                        )
                    for m in range(HL):
                        rope(
                            qT_sb[:, m * S + nb * 512 : m * S + (nb + 1) * 512],
                            qps[m][:],
                            nb * 512,
                            512,
                        )
                    rope(
                        kT_sb[:, nb * 512 : (nb + 1) * 512], kps[:], nb * 512, 512
                    )
                    # vT psum -> sbuf, then PE-transpose each 128-col chunk to
                    # natural [seq, hd] layout with a ones column appended.
                    vt = s1v.tile([128, 512], fp32, tag="vt")
                    nc.scalar.copy(vt[:], vps[:])
                    for j in range(4):
                        kcg = nb * 4 + j
                        vtp = ps_tr.tile([128, 128], fp32, tag="vtr")
                        nc.tensor.transpose(
                            vtp[:], vt[:, j * 128 : (j + 1) * 128], ident[:]
                        )
                        nc.scalar.copy(
                            v_sb[:, kcg * VST : kcg * VST + 128], vtp[:]
                        )
                        nc.vector.memset(
                            v_sb[:, kcg * VST + 128 : kcg * VST + 129], 1.0
                        )

            # ---------------- Stage 2: SDPA per head + AllToAll ----------------
            with (
                tc.tile_pool(name="sd", bufs=1) as sd,
                tc.tile_pool(name="sds", bufs=2) as sds,
                tc.tile_pool(name="msk", bufs=4) as mskp,
                tc.tile_pool(name="ps_l", bufs=1, space="PSUM") as ps_l,
                tc.tile_pool(name="ps_o", bufs=2, space="PSUM") as ps_o,
            ):
                for h in range(HL):
                    et = sd.tile([128, KC * S], bf16, tag="et")
                    # -- logitsT + exp, one [128, width] strip per k-chunk
                    for kc in range(KC):
                        qb_d = (kc * 128) // 512 if causal else 0
                        q_lo = qb_d * 512
                        width = S - q_lo
                        pl = ps_l.tile([128, S], fp32, tag="pl")
                        for qb in range(qb_d, QB):
                            fo = qb * 512 - q_lo
                            if (not causal) or qb == qb_d:
                                # mask (general) / zero-init handled by the
                                # first matmul's start flag below
                                pass
                            first = True
                            if not causal:
                                mt = mskp.tile([128, 512], fp32, tag="mt")
                                nc.sync.dma_start(
                                    mt[:],
                                    maskT[
                                        kc * 128 : (kc + 1) * 128,
                                        qb * 512 : (qb + 1) * 512,
                                    ],
                                )
                                nc.tensor.matmul(
                                    pl[:, fo : fo + 512],
                                    lhsT=ident[:],
                                    rhs=mt[:],
                                    start=True,
                                    stop=False,
                                )
                                first = False
                            nc.tensor.matmul(
                                pl[:, fo : fo + 512],
                                lhsT=kT_sb[:, kc * 128 : (kc + 1) * 128],
                                rhs=qT_sb[:, h * S + qb * 512 : h * S + (qb + 1) * 512],
                                start=first,
                                stop=True,
                            )
                        eslice = et[:, kc * S : kc * S + width]
                        nc.scalar.activation(
                            eslice, pl[:, 0:width], AF.Exp
                        )
                        if causal:
                            # zero strictly-upper part of the diagonal tile:
                            # keep where q >= k: -p + f + (q_lo - kc*128) >= 0
                            nc.gpsimd.affine_select(
                                out=et[:, kc * S : kc * S + 512],
                                in_=et[:, kc * S : kc * S + 512],
                                pattern=[[1, 512]],
                                compare_op=OP.is_ge,
                                # 0.0 lowers to an unallocated Pool_zero reg
                                # under Bacc; 1e-40 is 0 in bf16.
                                fill=1e-40,
                                base=q_lo - kc * 128,
                                channel_multiplier=-1,
                            )
                    # -- p @ [v | 1] accumulation over k-chunks, per q-chunk
                    for qc in range(KC):
                        kc_hi = qc if causal else KC - 1
                        po = ps_o.tile([128, 129], fp32, tag="po")
                        for kc in range(kc_hi + 1):
                            q_lo = ((kc * 128) // 512) * 512 if causal else 0
                            nc.tensor.matmul(
                                po[:],
                                lhsT=et[:, kc * S + qc * 128 - q_lo : kc * S + qc * 128 - q_lo + 128],
                                rhs=v_sb[:, kc * VST : kc * VST + 129],
                                start=(kc == 0),
                                stop=(kc == kc_hi),
                            )
                        rc = sds.tile([128, 1], fp32, tag="rc")
                        nc.vector.reciprocal(rc[:], po[:, 128:129])
                        osb = sds.tile([128, 128], fp32, tag="osb")
                        nc.vector.tensor_scalar_mul(osb[:], po[:, 0:128], rc[:])
                        otp = ps_o.tile([128, 128], fp32, tag="otp")
                        nc.tensor.transpose(otp[:], osb[:], ident[:])
                        ots = sds.tile([128, 128], fp32, tag="ots")
                        nc.scalar.copy(ots[:], otp[:])
                        nc.sync.dma_start(
                            a2a_in[h][
                                (qc // 2) * 128 : (qc // 2 + 1) * 128,
                                (qc % 2) * 128 : (qc % 2 + 1) * 128,
                            ],
                            ots[:],
                        )
                    nc.gpsimd.collective_compute(
                        "AllToAll",
                        OP.bypass,
                        replica_groups=rg,
                        ins=[a2a_in[h][:].opt()],
                        outs=[a2a_out[h][:].opt()],
                    )

            # ---------------- Stage 3: output projection ----------------
            with (
                tc.tile_pool(name="wo", bufs=8) as wop,
                tc.tile_pool(name="wolh", bufs=1) as wolh,
                tc.tile_pool(name="woob", bufs=2) as woob,
                tc.tile_pool(name="ps_w", bufs=2, space="PSUM") as ps_w,
            ):
                lh_sb = wolh.tile([128, DC * SQ], fp32, tag="lh")
                for r in range(NCORES):
                    for h in range(HL):
                        g = r * HL + h
                        nc.sync.dma_start(
                            lh_sb[:, g * SQ : (g + 1) * SQ],
                            a2a_out[h][r * 128 : (r + 1) * 128, :],
                        )
                for nbo in range(D // 512):
                    pw = [
                        ps_w.tile([128, 512], fp32, tag=f"wo{m}", name=f"pw{m}")
                        for m in range(2)
                    ]
                    for c in range(DC):
                        wt = wop.tile([128, 512], fp32, tag="wt")
                        nc.sync.dma_start(
                            wt[:],
                            woT[c * 128 : (c + 1) * 128, nbo * 512 : (nbo + 1) * 512],
                        )
                        for m in range(2):
                            nc.tensor.matmul(
                                pw[m][:],
                                lhsT=lh_sb[:, c * SQ + m * 128 : c * SQ + (m + 1) * 128],
                                rhs=wt[:],
                                start=(c == 0),
                                stop=(c == DC - 1),
                            )
                    for m in range(2):
                        ob = woob.tile([128, 512], fp32, tag="ob")
                        nc.scalar.copy(ob[:], pw[m][:])
                        nc.sync.dma_start(
                            out[m * 128 : (m + 1) * 128, nbo * 512 : (nbo + 1) * 512],
                            ob[:],
                        )
    nc.compile()
    return nc


_PERM = np.concatenate([np.arange(0, HD, 2), np.arange(1, HD, 2)])


def _stage_inputs(x, wq, wk, wv, wo, mask, freqs_cos, freqs_sin, causal):
    alpha = float(HD) ** -0.25  # sqrt of logit scale folded into both ropes
    xTc = np.ascontiguousarray(x.T)
    woTc = np.ascontiguousarray(wo.T)
    cosTc = np.ascontiguousarray(freqs_cos.T * alpha)
    sinTc = np.ascontiguousarray(freqs_sin.T * alpha)
    if not causal:
        maskTc = np.ascontiguousarray(np.maximum(mask, -60.0).T)
    in_maps = []
    for i in range(NCORES):
        wq_i = wq[i * DL : (i + 1) * DL, :].reshape(HL, HD, D)[:, _PERM, :]
        wk_i = wk[i * HD : (i + 1) * HD, :][_PERM, :]
        wv_i = wv[i * HD : (i + 1) * HD, :]
        m = dict(
            xT=xTc,
            wqT=np.ascontiguousarray(wq_i.reshape(DL, D).T),
            wkT=np.ascontiguousarray(wk_i.T),
            wvT=np.ascontiguousarray(wv_i.T),
            cosT=cosTc,
            sinT=sinTc,
            woT=woTc,
        )
        if not causal:
            m["maskT"] = maskTc
        in_maps.append(m)
    return in_maps


def _is_causal(mask):
    if mask.shape != (S, S):
        return False
    tri = np.tril(np.ones((S, S), bool))
    return bool(
        np.all(mask[tri] == 0.0) and np.all(mask[~tri] <= -1e8)
    )


def run(inputs, trace=False):
    from concourse.bass_utils import run_bass_kernel_spmd

    causal = _is_causal(np.asarray(inputs["mask"]))
    if causal not in _built:
        _built[causal] = _build(causal)
    nc = _built[causal]
    in_maps = _stage_inputs(
        np.asarray(inputs["x"], np.float32),
        np.asarray(inputs["wq"], np.float32),
        np.asarray(inputs["wk"], np.float32),
        np.asarray(inputs["wv"], np.float32),
        np.asarray(inputs["wo"], np.float32),
        np.asarray(inputs["mask"], np.float32),
        np.asarray(inputs["freqs_cos"], np.float32),
        np.asarray(inputs["freqs_sin"], np.float32),
        causal,
    )
    res = run_bass_kernel_spmd(
        nc, in_maps, core_ids=list(range(NCORES)), trace=trace
    )
    out = np.concatenate([res.results[i]["out"] for i in range(NCORES)], axis=0)
    return out, res


def kernel(**inputs):
    out, _ = run(inputs, trace=False)
    return out
